# revision 31
# baseline (speedup 1.0000x reference)
"""NT-Xent (SimCLR contrastive) loss on Trainium2, sharded across 8 NeuronCores.

Each core computes a [512, 4096] row-slice of the similarity matrix. The host
ships ONE rotated z^T per core (bf16, np.roll along columns so every core's
own rows land at columns 0-511 and partners at 2048-2559 — fully uniform
SPMD, no per-core side inputs). Per-core scalar partials are summed on host.

Per 1024-column block, pipelined (block b+1's norm emitted before block b's
Gram so the TensorE stream stays dense):
  - squares (DVE bf16 2x, in halves); column ssq via all-ones matmul
    (partition-reduce with free broadcast); rinv16 = exp(-0.5*ln(ssq) + ln S)
    [one ACT table set]
  - zn16 = z * rinv16 (DVE bf16 2x, halves), SWDGE cast-DMA -> fp8e4
    (truncation compensated by folding ~half an e4m3 ULP into S)
  - Gram block slice: fp8 DoubleRow matmuls; exp row-sums fused into the
    ScalarE activation accumulator
  - diagonal exact-recompute (gpsimd product + ones-matmul, matches the Gram
    diagonal) subtracted before the final ln
  - finale partition-reduce via gpsimd.partition_all_reduce
"""

import numpy as np

B = 2048
D = 512
N2 = 2 * B
NCORES = 8
RPC = N2 // NCORES      # 512
KT = D // 128           # 4
BLK = 1024
NBLK = N2 // BLK        # 4
TEMP = 0.1
SCALE = 1.0 / TEMP
FP8_SCALE = 16.0
SWDGE_CAST = True
TRUNC_COMP = 1.045 if SWDGE_CAST else 1.0
LN_S = float(np.log(FP8_SCALE * TRUNC_COMP))
# ssq via fp8-DR matmuls (b>0): squares are also truncation-cast, so fold
# half the same compensation into the rinv bias.
LN_S_SQ = LN_S - 0.5 * float(np.log(1.045))
EXP_SCALE = SCALE / (FP8_SCALE * FP8_SCALE)
PR_GPSIMD = False       # gpsimd queue must stay clear for SWDGE cast issues

_CACHE = {}


def _patch_act_tables(nc, mybir):
    """Make Ln and Exp resolve to the shared natural_log_exp_and_others set
    so the compiler emits one ACT table load instead of thrashing."""
    from concourse import hw_specs

    tables = hw_specs.get_activation_tables(nc.m.arch)
    keep = "natural_log_exp_and_others"
    if keep not in tables:
        return
    F = mybir.ActivationFunctionType
    if F.Exp not in tables[keep] or F.Ln not in tables[keep]:
        return
    for name, fns in tables.items():
        if name != keep:
            fns.discard(F.Exp)
            fns.discard(F.Ln)


def _build():
    from concourse import bass, bacc, tile, mybir, bass_isa

    nc = bacc.Bacc("TRN2", target_bir_lowering=False, debug=False,
                   num_devices=NCORES)
    bf16 = mybir.dt.bfloat16
    f32 = mybir.dt.float32
    f8 = mybir.dt.float8e4
    F = mybir.ActivationFunctionType
    A = mybir.AluOpType
    AX = mybir.AxisListType
    DR = mybir.MatmulPerfMode.DoubleRow
    PSUM = bass.MemorySpace.PSUM

    zt = nc.dram_tensor("zt", [D, N2], bf16, kind="ExternalInput").ap()
    out = nc.dram_tensor("out", [1, 1], f32, kind="ExternalOutput").ap()

    with tile.TileContext(nc) as tc:
        with (
            tc.tile_pool(name="sb", bufs=1) as sb,
            tc.tile_pool(name="wz", bufs=4) as wz,
            tc.tile_pool(name="wq", bufs=2) as wq,
            tc.tile_pool(name="wn", bufs=2) as wn,
            tc.tile_pool(name="wr", bufs=2) as wr,
            tc.tile_pool(name="psA", bufs=2, space=PSUM) as psA,
            tc.tile_pool(name="psB", bufs=2, space=PSUM) as psB,
        ):
            ones = sb.tile([128, 128], bf16, tag="ones")
            nc.vector.memset(ones[:], 1.0)
            ones2 = sb.tile([128, 2, 128], f8, tag="ones2")
            nc.vector.memset(ones2[:], 1.0)
            bias_lnSQ = sb.tile([128, 1], f32, tag="blnSQ")
            nc.vector.memset(bias_lnSQ[:], LN_S_SQ)
            bias_ln16 = sb.tile([128, 1], f32, tag="bln16")
            nc.vector.memset(bias_ln16[:], float(np.log(FP8_SCALE)))
            bias_10 = sb.tile([128, 1], f32, tag="b10")
            nc.vector.memset(bias_10[:], SCALE)
            rowp = sb.tile([128, 4, NBLK], f32, tag="rowp")

            zn8_t = [None] * NBLK
            zb_t = [None] * NBLK

            def load_block(b):
                zb = wz.tile([128, KT, BLK], bf16, tag="zb")
                bsl = slice(b * BLK, (b + 1) * BLK)
                for k in range(KT):
                    eng = nc.sync if k % 2 == 0 else nc.scalar
                    eng.dma_start(out=zb[:, k, :],
                                  in_=zt[k * 128:(k + 1) * 128, bsl])
                zb_t[b] = zb

            def norm_block(b):
                zb = zb_t[b]
                sq = wq.tile([128, KT, BLK], bf16, tag="sq")
                ssq = psA.tile([128, BLK], f32, tag="ps1")
                # trickle: bridge the PE p-state across the sq wait
                for _ in range(2):
                    nc.tensor.matmul(ssq[:, 0:128], ones[:], ones[:],
                                     start=True, stop=True)
                if b == 0:
                    # bf16 ssq (short chain for pipeline fill)
                    for h in range(2):
                        nc.vector.tensor_tensor(
                            sq[:, 2 * h:2 * h + 2, :],
                            zb[:, 2 * h:2 * h + 2, :],
                            zb[:, 2 * h:2 * h + 2, :], A.mult)
                        for k in (2 * h, 2 * h + 1):
                            for j in range(BLK // 512):
                                nc.tensor.matmul(
                                    ssq[:, j * 512:(j + 1) * 512], ones[:],
                                    sq[:, k, j * 512:(j + 1) * 512],
                                    start=(k == 0), stop=(k == KT - 1))
                else:
                    # fp8 DoubleRow ssq: 4 matmuls instead of 16
                    sq8 = wq.tile([128, KT, BLK], f8, tag="sq8")
                    for h in range(2):
                        nc.vector.tensor_tensor(
                            sq[:, 2 * h:2 * h + 2, :],
                            zb[:, 2 * h:2 * h + 2, :],
                            zb[:, 2 * h:2 * h + 2, :], A.mult)
                        nc.gpsimd.dma_start(out=sq8[:, 2 * h:2 * h + 2, :],
                                            in_=sq[:, 2 * h:2 * h + 2, :])
                    for h in range(2):
                        for j in range(BLK // 512):
                            nc.tensor.matmul(
                                ssq[:, j * 512:(j + 1) * 512], ones2[:],
                                sq8[:, 2 * h:2 * h + 2, j * 512:(j + 1) * 512],
                                start=(h == 0), stop=(h == 1),
                                perf_mode=DR)
                lns = wr.tile([128, BLK], f32, tag="lns")
                nc.scalar.activation(lns[:], ssq[:], F.Ln)
                rb = wr.tile([128, BLK], bf16, tag="rb")
                bias = bias_lnSQ[:] if b > 0 else bias_ln16[:]
                nc.scalar.activation(rb[:], lns[:], F.Exp, scale=-0.5,
                                     bias=bias)
                zn8 = sb.tile([128, KT, BLK], f8, tag=f"zn8_{b}")
                if b == 0:
                    # fill fast path: DVE writes fp8 directly (1x rate but no
                    # cast-DMA latency before the very first Gram)
                    for k in range(KT):
                        nc.vector.tensor_tensor(zn8[:, k, :], zb[:, k, :],
                                                rb[:], A.mult)
                else:
                    zn16 = wn.tile([128, KT, BLK], bf16, tag="zn16")
                    for h in range(2):
                        nc.vector.tensor_tensor(
                            zn16[:, 2 * h:2 * h + 2, :],
                            zb[:, 2 * h:2 * h + 2, :],
                            rb[:, None, :].to_broadcast((128, 2, BLK)), A.mult)
                        nc.gpsimd.dma_start(out=zn8[:, 2 * h:2 * h + 2, :],
                                            in_=zn16[:, 2 * h:2 * h + 2, :])
                zn8_t[b] = zn8

            def gram_block(b):
                zn8 = zn8_t[b]
                zn80 = zn8_t[0]
                for m in range(4):
                    pm = psB.tile([128, BLK], f32, tag="pm")
                    if m == 0:
                        # trickle: bridge the PE p-state across the cast wait
                        for _ in range(2):
                            nc.tensor.matmul(pm[:, 0:128], ones[:], ones[:],
                                             start=True, stop=True)
                    for g in range(2):
                        lhsT = zn80[:, 2 * g:2 * g + 2, m * 128:(m + 1) * 128]
                        for j in range(BLK // 512):
                            nc.tensor.matmul(
                                pm[:, j * 512:(j + 1) * 512], lhsT,
                                zn8[:, 2 * g:2 * g + 2, j * 512:(j + 1) * 512],
                                start=(g == 0), stop=(g == 1), perf_mode=DR)
                    scr = wn.tile([128, BLK], bf16, tag="scr")
                    nc.scalar.activation(scr[:], pm[:], F.Exp, scale=EXP_SCALE,
                                         accum_out=rowp[:, m, b:b + 1])

            def diag_path():
                zn80 = zn8_t[0]
                prd = wq.tile([128, KT, RPC], bf16, tag="prd")
                eng = nc.gpsimd if PR_GPSIMD else nc.vector
                eng.tensor_tensor(prd[:], zn80[:, :, 0:RPC],
                                  zn80[:, :, 0:RPC], A.mult)
                dgt = psA.tile([128, BLK], f32, tag="ps1")
                for k in range(KT):
                    nc.tensor.matmul(dgt[0:1, 0:RPC], ones[:, 0:1],
                                     prd[:, k, :],
                                     start=(k == 0), stop=(k == KT - 1))
                diag_row = sb.tile([1, RPC], bf16, tag="diagrow")
                nc.vector.tensor_scalar_add(diag_row[:], dgt[0:1, 0:RPC],
                                            -FP8_SCALE ** 2)
                return diag_row

            def pos_path():
                prp = wq.tile([128, KT, RPC], bf16, tag="prp")
                eng = nc.gpsimd if PR_GPSIMD else nc.vector
                eng.tensor_tensor(prp[:], zn8_t[0][:, :, 0:RPC],
                                  zn8_t[2][:, :, 0:RPC], A.mult)
                pp = psA.tile([128, BLK], f32, tag="ps1")
                for k in range(KT):
                    nc.tensor.matmul(pp[:, 0:RPC], ones[:], prp[:, k, :],
                                     start=(k == 0), stop=(k == KT - 1))
                pos_red = sb.tile([128, 1], f32, tag="posr")
                nc.vector.tensor_reduce(pos_red[:], pp[:, 0:RPC], AX.X, A.add)
                return pos_red

            # ---- prologue ----
            warm = psA.tile([128, BLK], f32, tag="ps1")
            for _ in range(16):
                nc.tensor.matmul(warm[:, 0:128], ones[:], ones[:],
                                 start=True, stop=True)
            load_block(0)
            norm_block(0)
            load_block(1)

            diag_row = pos_red = None
            for b in range(NBLK):
                if b + 2 < NBLK:
                    load_block(b + 2)
                if b + 1 < NBLK:
                    norm_block(b + 1)
                gram_block(b)
                if b == 0:
                    diag_row = diag_path()
                if b == 2:
                    pos_red = pos_path()

            # ---- finale ----
            dt = psA.tile([128, BLK], f32, tag="ps1")
            for m in range(4):
                nc.tensor.matmul(dt[:, m:m + 1],
                                 diag_row[0:1, m * 128:(m + 1) * 128],
                                 ones[0:1, 0:1], start=True, stop=True)
            diag_part = sb.tile([128, 4], f32, tag="diagp")
            nc.vector.tensor_copy(diag_part[:], dt[:, 0:4])
            dexp = sb.tile([128, 4], f32, tag="dexp")
            nc.scalar.activation(dexp[:], diag_part[:], F.Exp, scale=EXP_SCALE,
                                 bias=bias_10[:])
            zsum = sb.tile([128, 4, 1], f32, tag="zsum")
            nc.vector.tensor_reduce(zsum[:], rowp[:], AX.X, A.add)
            zarg = sb.tile([128, 4], f32, tag="zarg")
            nc.vector.tensor_tensor(zarg[:], zsum[:, :, 0], dexp[:],
                                    A.subtract)
            logz = sb.tile([128, 5], f32, tag="logz")
            nc.scalar.activation(logz[:, 0:4], zarg[:], F.Ln)
            nc.vector.tensor_scalar_mul(
                logz[:, 4:5], pos_red[:],
                -SCALE / (FP8_SCALE ** 2) / 128.0)
            red1 = sb.tile([128, 1], f32, tag="red1")
            nc.vector.tensor_reduce(red1[:], logz[:], AX.X, A.add)
            fin = sb.tile([128, 1], f32, tag="fin")
            nc.gpsimd.partition_all_reduce(fin[:], red1[:], 128,
                                           bass_isa.ReduceOp.add)
            nc.sync.dma_start(out=out, in_=fin[0:1, :])

    _patch_act_tables(nc, mybir)
    nc.compile()
    return nc


def _get_nc():
    if "nc" not in _CACHE:
        _CACHE["nc"] = _build()
    return _CACHE["nc"]


def _in_maps(z_i, z_j):
    import ml_dtypes

    z = np.concatenate(
        [np.asarray(z_i, np.float32), np.asarray(z_j, np.float32)], axis=0)
    zt = np.ascontiguousarray(z.T).astype(ml_dtypes.bfloat16)
    return [{"zt": np.ascontiguousarray(np.roll(zt, -c * RPC, axis=1))}
            for c in range(NCORES)]


def _run(z_i, z_j, trace=False):
    from concourse.bass_utils import run_bass_kernel_spmd

    nc = _get_nc()
    return run_bass_kernel_spmd(nc, _in_maps(z_i, z_j), list(range(NCORES)),
                                trace=trace)


def kernel(z_i, z_j):
    res = _run(z_i, z_j, trace=False)
    total = sum(float(r["out"][0, 0]) for r in res.results)
    return np.float32(total / N2)
